# revision 17
# baseline (speedup 1.0000x reference)
"""Causal self-attention (B=2, T=2048, D=2048, H=16) on 8 TRN2 NeuronCores.

Sharding: 2-way batch-parallel x 4-way head-parallel. Core c owns batch
c//4 and heads 4*(c%4)..4*(c%4)+3. Each core computes its heads'
contribution through the output projection for its batch; the host sums
the 4 partials per batch.

Per-core kernel (bf16 compute, f32 PSUM accumulation):
  P1  QKV projection over t-chunks of 512: Q^T/K^T produced in [hd, t]
      layout (rope "split" order: re pairs in partitions 0-63, im in
      64-127) via wqk as stationary; rope applied with 4 DVE ops reading
      the PSUM tile directly; V natural [t, hd] per head (ACT copy).
      x loads ride the scalar-engine HWDGE ring so they issue in
      parallel with the sync-ring weight loads at startup.
  P2  Flash-style causal attention per (q-chunk, head) in S^T layout:
      S^T = K_blk @ Q^T (PSUM), E^T = exp(scale*S^T) via ACT (bf16),
      then two accumulating matmuls per s-block: O^T += V_blk.T @ E^T
      (N=512, output already transposed) and L += ones.T @ E^T
      (all-ones stationary -> every partition row of L is the softmax
      denominator, i.e. a free broadcast). Diagonal s-blocks compute
      only the live q-column suffix and apply a single reusable 128x128
      triangle mask. Normalize: reciprocal_approx_fast(L), then
      O^T * (1/L) -> attnT.
  P3  Output projection interleaved per q-chunk: attnT x wp with 4
      head-blocks as contraction steps; PSUM->SBUF copies split between
      ACT and DVE; stores overlap compute.
"""
import numpy as np

import concourse.bass as bass
from concourse import bacc
import concourse.tile as tile
from concourse import mybir
from concourse.bass_utils import run_bass_kernel_spmd

B, T, D, H = 2, 2048, 2048, 16
HD = D // H            # 128
HPC = 4                # heads per core
FL = HPC * HD          # local features = 512
TC = 512               # t-chunk for QKV phase
NTC = T // TC          # 4
QC = 512               # q-chunk for attention
NQC = T // QC          # 4
SCALE = float(1.0 / np.sqrt(np.float32(HD)))

f32 = mybir.dt.float32
bf16 = mybir.dt.bfloat16

_BUILT = {}


def _build_nc():
    nc = bacc.Bacc()

    xT = nc.dram_tensor("xT", (D, T), bf16, kind="ExternalInput")
    wqk = nc.dram_tensor("wqk", (D, 2 * FL), bf16, kind="ExternalInput")
    wv = nc.dram_tensor("wv", (D, FL), bf16, kind="ExternalInput")
    wp = nc.dram_tensor("wp", (HPC, HD, D), bf16, kind="ExternalInput")
    cos2 = nc.dram_tensor("cos2", (HD, T), bf16, kind="ExternalInput")
    sinn = nc.dram_tensor("sinn", (HD, T), bf16, kind="ExternalInput")
    dmask = nc.dram_tensor("dmask", (128, 128), bf16, kind="ExternalInput")
    onesd = nc.dram_tensor("onesd", (128, 128), bf16, kind="ExternalInput")
    outp = nc.dram_tensor("outp", (T, D), f32, kind="ExternalOutput")
    tick = nc.dram_tensor("tick", (128, 8), f32, kind="ExternalInput")
    tock = nc.dram_tensor("tock", (128, 8), f32, kind="ExternalOutput")

    with tile.TileContext(nc) as tc:
        from contextlib import ExitStack
        with ExitStack() as top:
            pers = top.enter_context(tc.tile_pool(name="pers", bufs=1))
            # persistent: weights, rope multipliers, masks, ones
            wqk_sb = [pers.tile([128, 2 * FL], bf16, tag=f"wqk{k}", name=f"wqk{k}")
                      for k in range(16)]
            cos_sb = pers.tile([HD, T], bf16)
            sin_sb = pers.tile([HD, T], bf16)
            wv_sb = [pers.tile([128, FL], bf16, tag=f"wv{k}", name=f"wv{k}")
                     for k in range(16)]
            wp_sb = [pers.tile([128, D], bf16, tag=f"wp{h}", name=f"wp{h}")
                     for h in range(HPC)]
            tri_sb = pers.tile([128, 128], bf16)
            ones_sb = pers.tile([128, 128], bf16)
            # per-core persistent activations
            qT = [pers.tile([HD, T], bf16, tag=f"qT{h}", name=f"qT{h}")
                  for h in range(HPC)]
            kT = [pers.tile([HD, T], bf16, tag=f"kT{h}", name=f"kT{h}")
                  for h in range(HPC)]
            vP = [pers.tile([128, FL], bf16, tag=f"vP{i}", name=f"vP{i}")
                  for i in range(T // 128)]
            aT = [pers.tile([128, T], bf16, tag=f"aT{h}", name=f"aT{h}")
                  for h in range(HPC)]

            # DMA order matters for startup: wqk + cos/sin first (P1 chunk 0
            # needs them), wv next, wp/masks last (P2/P3 only).
            for k in range(16):
                nc.sync.dma_start(out=wqk_sb[k], in_=wqk[k * 128:(k + 1) * 128, :])
            nc.sync.dma_start(out=cos_sb, in_=cos2[:, :])
            nc.sync.dma_start(out=sin_sb, in_=sinn[:, :])
            for k in range(16):
                nc.sync.dma_start(out=wv_sb[k], in_=wv[k * 128:(k + 1) * 128, :])
            tick_sb = pers.tile([128, 8], f32)
            nc.sync.dma_start(out=tick_sb, in_=tick[:, :])
            nc.sync.dma_start(out=tock[:, :], in_=tick_sb)
            for h in range(HPC):
                nc.sync.dma_start(out=wp_sb[h], in_=wp[h, :, :])
            nc.sync.dma_start(out=tri_sb, in_=dmask[:, :])
            nc.sync.dma_start(out=ones_sb, in_=onesd[:, :])

            # ---- P1: QKV + rope ----
            with ExitStack() as p1:
                xp = p1.enter_context(tc.tile_pool(name="xp", bufs=20))
                rp = p1.enter_context(tc.tile_pool(name="rp", bufs=6))
                ps_qk = p1.enter_context(
                    tc.tile_pool(name="psqk", bufs=3, space="PSUM"))
                ps_v = p1.enter_context(
                    tc.tile_pool(name="psv", bufs=2, space="PSUM"))
                for t in range(NTC):
                    t0 = t * TC
                    if t % 2 == 0:
                        # one DMA covers a chunk-pair (2*TC columns)
                        xt2 = []
                        for k in range(16):
                            xk = xp.tile([128, 2 * TC], bf16, tag="xt", name="xt")
                            # scalar-ring HWDGE: parallel issue with the
                            # sync-ring weight loads at startup
                            nc.scalar.dma_start(
                                out=xk,
                                in_=xT[k * 128:(k + 1) * 128, t0:t0 + 2 * TC])
                            xt2.append(xk)
                    half = (t % 2) * TC
                    xt = [xk[:, half:half + TC] for xk in xt2]
                    # Q^T/K^T accumulate: 8 f-blocks (q_h0..q_h3, k_h0..k_h3)
                    for fb in range(8):
                        ps = ps_qk.tile([128, TC], f32, tag="psqk")
                        for k in range(16):
                            nc.tensor.matmul(
                                ps[:, :],
                                wqk_sb[k][:, fb * 128:(fb + 1) * 128],
                                xt[k][:, :],
                                start=(k == 0), stop=(k == 15))
                        # rope from psum -> SBUF qT/kT slice
                        dst = (qT[fb] if fb < HPC else kT[fb - HPC])
                        dsl = dst[:, t0:t0 + TC]
                        ca = cos_sb[:, t0:t0 + TC]
                        sa = sin_sb[:, t0:t0 + TC]
                        m1 = rp.tile([128, TC], bf16, tag="m1")
                        m2 = rp.tile([128, TC], bf16, tag="m2")
                        nc.vector.tensor_mul(m1, ps[:, :], ca)
                        nc.vector.tensor_mul(
                            m2[0:64, :], ps[64:128, :], sa[0:64, :])
                        nc.vector.tensor_mul(
                            m2[64:128, :], ps[0:64, :], sa[64:128, :])
                        nc.vector.tensor_add(dsl, m1, m2)
                    # V natural accumulate: TC/128 t-blocks
                    for tb in range(TC // 128):
                        ps = ps_v.tile([128, FL], f32, tag="psv")
                        for k in range(16):
                            nc.tensor.matmul(
                                ps[:, :],
                                xt[k][:, tb * 128:(tb + 1) * 128],
                                wv_sb[k][:, :],
                                start=(k == 0), stop=(k == 15))
                        nc.scalar.copy(vP[(t0 + tb * 128) // 128], ps[:, :])

            # ---- P2+P3 interleaved per q-chunk ----
            with ExitStack() as p2:
                etp = p2.enter_context(tc.tile_pool(name="etp", bufs=8))
                lvp = p2.enter_context(tc.tile_pool(name="lvp", bufs=3))
                op = p2.enter_context(tc.tile_pool(name="op", bufs=4))
                ps_s = p2.enter_context(
                    tc.tile_pool(name="pss", bufs=2, space="PSUM"))
                ps_o = p2.enter_context(
                    tc.tile_pool(name="pso", bufs=2, space="PSUM"))
                ps_l = p2.enter_context(
                    tc.tile_pool(name="psl", bufs=2, space="PSUM"))
                ps_p = p2.enter_context(
                    tc.tile_pool(name="psp", bufs=2, space="PSUM"))
                for qc in reversed(range(NQC)):
                    q0 = qc * QC
                    nsb = (qc + 1) * (QC // 128)  # causal: s-blocks <= q-chunk end
                    for h in range(HPC):
                        po = ps_o.tile([128, QC], f32, tag="pso")
                        pl = ps_l.tile([128, QC], f32, tag="psl")
                        for sb in range(nsb):
                            s0 = sb * 128
                            # causal: only q-columns >= s0 are live in this
                            # s-block; skip the dead prefix entirely
                            j0 = max(0, s0 - q0)
                            ps = ps_s.tile([128, QC], f32, tag="pss")
                            nc.tensor.matmul(
                                ps[:, j0:QC],
                                kT[h][:, s0:s0 + 128],
                                qT[h][:, q0 + j0:q0 + QC],
                                start=True, stop=True)
                            et = etp.tile([128, QC], bf16, tag="et")
                            nc.scalar.activation(
                                et[:, j0:QC], ps[:, j0:QC],
                                mybir.ActivationFunctionType.Exp, scale=SCALE)
                            if s0 >= q0:  # intra-block triangle mask
                                nc.vector.tensor_mul(
                                    et[:, j0:j0 + 128], et[:, j0:j0 + 128],
                                    tri_sb)
                            nc.tensor.matmul(
                                po[:, j0:QC], vP[sb][:, h * HD:(h + 1) * HD],
                                et[:, j0:QC],
                                start=(sb == 0), stop=(sb == nsb - 1))
                            nc.tensor.matmul(
                                pl[:, j0:QC], ones_sb, et[:, j0:QC],
                                start=(sb == 0), stop=(sb == nsb - 1))
                        linv = lvp.tile([128, QC], f32, tag="linv")
                        nc.vector.reciprocal_approx_fast(out=linv, in_=pl[:, :])
                        nc.vector.tensor_mul(
                            aT[h][:, q0:q0 + QC], po[:, :], linv)
                    # P3 for this q-chunk's t-range; ec-pairs share one
                    # [128, 1024] staging tile and a single store DMA
                    for tb in range(QC // 128):
                        t0 = q0 + tb * 128
                        for ecp in range(2):
                            ot = op.tile([128, 1024], f32, tag="ot")
                            for eh in range(2):
                                ec = ecp * 2 + eh
                                psp = ps_p.tile([128, 512], f32, tag="psp")
                                for h in range(HPC):
                                    nc.tensor.matmul(
                                        psp[:, :],
                                        aT[h][:, t0:t0 + 128],
                                        wp_sb[h][:, ec * 512:(ec + 1) * 512],
                                        start=(h == 0), stop=(h == HPC - 1))
                                if eh == 0:
                                    nc.scalar.copy(
                                        ot[:, 0:512], psp[:, :])
                                else:
                                    nc.vector.tensor_copy(
                                        ot[:, 512:1024], psp[:, :])
                            nc.sync.dma_start(
                                out=outp[t0:t0 + 128,
                                         ecp * 1024:(ecp + 1) * 1024],
                                in_=ot)
    nc.finalize()
    return nc


def _prep_in_maps(x, rope, mask, w_attn, w_proj):
    import ml_dtypes
    bf = ml_dtypes.bfloat16
    x = np.asarray(x, dtype=np.float32)
    rope = np.asarray(rope, dtype=np.float32)
    mask = np.asarray(mask)
    w_attn = np.asarray(w_attn, dtype=np.float32)
    w_proj = np.asarray(w_proj, dtype=np.float32)

    xTb = [np.ascontiguousarray(x[b].T).astype(bf) for b in range(B)]
    cosT = np.ascontiguousarray(rope[:, :, 0].T)  # (64, T)
    sinT = np.ascontiguousarray(rope[:, :, 1].T)
    cos2 = np.concatenate([cosT, cosT], axis=0).astype(bf)    # (128, T)
    sinn = np.concatenate([-sinT, sinT], axis=0).astype(bf)   # (128, T)

    # intra-block triangle: tri[i, j] = mask[q=j, s=i] over a 128 window
    m128 = np.asarray(mask[0, 0, :128, :128])
    dm = np.ascontiguousarray(m128.T.astype(np.float32)).astype(bf)

    perm = np.concatenate([np.arange(0, HD, 2), np.arange(1, HD, 2)])
    in_maps = []
    for c in range(8):
        b = c // 4
        heads = [4 * (c % 4) + j for j in range(HPC)]
        qrows = np.concatenate([w_attn[h * HD:(h + 1) * HD][perm] for h in heads])
        krows = np.concatenate(
            [w_attn[D + h * HD:D + (h + 1) * HD][perm] for h in heads])
        vrows = np.concatenate(
            [w_attn[2 * D + h * HD:2 * D + (h + 1) * HD] for h in heads])
        wqk_m = np.ascontiguousarray(
            np.concatenate([qrows, krows]).T).astype(bf)     # (D, 1024)
        wv_m = np.ascontiguousarray(vrows.T).astype(bf)      # (D, 512)
        wp_m = np.stack(
            [np.ascontiguousarray(w_proj[:, h * HD:(h + 1) * HD].T)
             for h in heads]).astype(bf)                     # (4, 128, D)
        im = {"xT": xTb[b], "wqk": wqk_m, "wv": wv_m, "wp": wp_m,
              "cos2": cos2, "sinn": sinn, "dmask": dm,
              "onesd": np.ones((128, 128), dtype=np.float32).astype(bf),
              "tick": np.zeros((128, 8), np.float32)}
        in_maps.append(im)
    return in_maps


def kernel(x, rope, mask, w_attn, w_proj):
    if "nc" not in _BUILT:
        _BUILT["nc"] = _build_nc()
    nc = _BUILT["nc"]
    in_maps = _prep_in_maps(x, rope, mask, w_attn, w_proj)
    res = run_bass_kernel_spmd(nc, in_maps, core_ids=list(range(8)))
    out = np.zeros((B, T, D), dtype=np.float64)
    for c in range(8):
        out[c // 4] += res.results[c]["outp"].astype(np.float64)
    return out.astype(np.float32)


# revision 18
# speedup vs baseline: 1.0913x; 1.0913x over previous
"""Causal self-attention (B=2, T=2048, D=2048, H=16) on 8 TRN2 NeuronCores.

Sharding: 2-way batch-parallel x 4-way head-parallel. Core c owns batch
c//4 and heads 4*(c%4)..4*(c%4)+3. Each core computes its heads'
contribution through the output projection for its batch; the host sums
the 4 partials per batch.

Per-core kernel (bf16 compute, f32 PSUM accumulation):
  P1  QKV projection over t-chunks of 512: Q^T/K^T produced in [hd, t]
      layout (rope "split" order: re pairs in partitions 0-63, im in
      64-127) via wqk as stationary; rope applied with 4 DVE ops reading
      the PSUM tile directly; V natural [t, hd] per head (ACT copy).
      x loads ride the scalar-engine HWDGE ring so they issue in
      parallel with the sync-ring weight loads at startup.
  P2  Flash-style causal attention per (q-chunk, head) in S^T layout:
      S^T = K_blk @ Q^T (PSUM), E^T = exp(scale*S^T) via ACT (bf16),
      then two accumulating matmuls per s-block: O^T += V_blk.T @ E^T
      (N=512, output already transposed) and L += ones.T @ E^T
      (all-ones stationary -> every partition row of L is the softmax
      denominator, i.e. a free broadcast). Diagonal s-blocks compute
      only the live q-column suffix and apply a single reusable 128x128
      triangle mask. Normalize: reciprocal_approx_fast(L), then
      O^T * (1/L) -> attnT.
  P3  Output projection interleaved per q-chunk: attnT x wp with 4
      head-blocks as contraction steps; PSUM->SBUF copies split between
      ACT and DVE; stores overlap compute.
"""
import numpy as np

import concourse.bass as bass
from concourse import bacc
import concourse.tile as tile
from concourse import mybir
from concourse.bass_utils import run_bass_kernel_spmd

B, T, D, H = 2, 2048, 2048, 16
HD = D // H            # 128
HPC = 4                # heads per core
FL = HPC * HD          # local features = 512
TC = 512               # t-chunk for QKV phase
NTC = T // TC          # 4
QC = 512               # q-chunk for attention
NQC = T // QC          # 4
SCALE = float(1.0 / np.sqrt(np.float32(HD)))

f32 = mybir.dt.float32
bf16 = mybir.dt.bfloat16

_BUILT = {}


def _build_nc():
    nc = bacc.Bacc()

    xT = nc.dram_tensor("xT", (D, T), bf16, kind="ExternalInput")
    wqkv = nc.dram_tensor("wqkv", (D, 3 * FL), bf16, kind="ExternalInput")
    wp = nc.dram_tensor("wp", (HD, HPC * D), bf16, kind="ExternalInput")
    cossin = nc.dram_tensor("cossin", (HD, 2 * T), bf16, kind="ExternalInput")
    trione = nc.dram_tensor("trione", (128, 256), bf16, kind="ExternalInput")
    outp = nc.dram_tensor("outp", (T, D), f32, kind="ExternalOutput")
    tick = nc.dram_tensor("tick", (128, 8), f32, kind="ExternalInput")
    tock = nc.dram_tensor("tock", (128, 8), f32, kind="ExternalOutput")

    with tile.TileContext(nc) as tc:
        from contextlib import ExitStack
        with ExitStack() as top:
            pers = top.enter_context(tc.tile_pool(name="pers", bufs=1))
            # persistent: weights, rope multipliers, masks, ones
            wqkv_sb = [pers.tile([128, 3 * FL], bf16, tag=f"wqkv{k}",
                                 name=f"wqkv{k}") for k in range(16)]
            cs_sb = pers.tile([HD, 2 * T], bf16)
            wp_sb = pers.tile([128, HPC * D], bf16)
            to_sb = pers.tile([128, 256], bf16)
            # per-core persistent activations
            qT = [pers.tile([HD, T], bf16, tag=f"qT{h}", name=f"qT{h}")
                  for h in range(HPC)]
            kT = [pers.tile([HD, T], bf16, tag=f"kT{h}", name=f"kT{h}")
                  for h in range(HPC)]
            vP = [pers.tile([128, FL], bf16, tag=f"vP{i}", name=f"vP{i}")
                  for i in range(T // 128)]
            aT = [pers.tile([128, T], bf16, tag=f"aT{h}", name=f"aT{h}")
                  for h in range(HPC)]

            # DMA order matters for startup: fused wqkv + cos/sin first
            # (P1 chunk 0 needs them); wp/masks last (P2/P3 only).
            for k in range(16):
                nc.sync.dma_start(out=wqkv_sb[k],
                                  in_=wqkv[k * 128:(k + 1) * 128, :])
            nc.sync.dma_start(out=cs_sb, in_=cossin[:, :])
            tick_sb = pers.tile([128, 8], f32)
            nc.sync.dma_start(out=tick_sb, in_=tick[:, :])
            nc.sync.dma_start(out=tock[:, :], in_=tick_sb)
            nc.sync.dma_start(out=wp_sb, in_=wp[:, :])
            nc.sync.dma_start(out=to_sb, in_=trione[:, :])
            tri_sb = to_sb[:, 0:128]
            ones_sb = to_sb[:, 128:256]

            # ---- P1: QKV + rope ----
            with ExitStack() as p1:
                xp = p1.enter_context(tc.tile_pool(name="xp", bufs=20))
                rp = p1.enter_context(tc.tile_pool(name="rp", bufs=6))
                ps_qk = p1.enter_context(
                    tc.tile_pool(name="psqk", bufs=3, space="PSUM"))
                ps_v = p1.enter_context(
                    tc.tile_pool(name="psv", bufs=2, space="PSUM"))
                for t in range(NTC):
                    t0 = t * TC
                    if t % 2 == 0:
                        # one DMA covers a chunk-pair (2*TC columns)
                        xt2 = []
                        for k in range(16):
                            xk = xp.tile([128, 2 * TC], bf16, tag="xt", name="xt")
                            # scalar-ring HWDGE: parallel issue with the
                            # sync-ring weight loads at startup
                            nc.scalar.dma_start(
                                out=xk,
                                in_=xT[k * 128:(k + 1) * 128, t0:t0 + 2 * TC])
                            xt2.append(xk)
                    half = (t % 2) * TC
                    xt = [xk[:, half:half + TC] for xk in xt2]
                    # Q^T/K^T accumulate: 8 f-blocks (q_h0..q_h3, k_h0..k_h3)
                    for fb in range(8):
                        ps = ps_qk.tile([128, TC], f32, tag="psqk")
                        for k in range(16):
                            nc.tensor.matmul(
                                ps[:, :],
                                wqkv_sb[k][:, fb * 128:(fb + 1) * 128],
                                xt[k][:, :],
                                start=(k == 0), stop=(k == 15))
                        # rope from psum -> SBUF qT/kT slice
                        dst = (qT[fb] if fb < HPC else kT[fb - HPC])
                        dsl = dst[:, t0:t0 + TC]
                        ca = cs_sb[:, t0:t0 + TC]
                        sa = cs_sb[:, T + t0:T + t0 + TC]
                        m1 = rp.tile([128, TC], bf16, tag="m1")
                        m2 = rp.tile([128, TC], bf16, tag="m2")
                        nc.vector.tensor_mul(m1, ps[:, :], ca)
                        nc.vector.tensor_mul(
                            m2[0:64, :], ps[64:128, :], sa[0:64, :])
                        nc.vector.tensor_mul(
                            m2[64:128, :], ps[0:64, :], sa[64:128, :])
                        nc.vector.tensor_add(dsl, m1, m2)
                    # V natural accumulate: TC/128 t-blocks
                    for tb in range(TC // 128):
                        ps = ps_v.tile([128, FL], f32, tag="psv")
                        for k in range(16):
                            nc.tensor.matmul(
                                ps[:, :],
                                xt[k][:, tb * 128:(tb + 1) * 128],
                                wqkv_sb[k][:, 2 * FL:3 * FL],
                                start=(k == 0), stop=(k == 15))
                        nc.scalar.copy(vP[(t0 + tb * 128) // 128], ps[:, :])

            # ---- P2+P3 interleaved per q-chunk ----
            with ExitStack() as p2:
                etp = p2.enter_context(tc.tile_pool(name="etp", bufs=8))
                lvp = p2.enter_context(tc.tile_pool(name="lvp", bufs=3))
                op = p2.enter_context(tc.tile_pool(name="op", bufs=4))
                ps_s = p2.enter_context(
                    tc.tile_pool(name="pss", bufs=2, space="PSUM"))
                ps_o = p2.enter_context(
                    tc.tile_pool(name="pso", bufs=2, space="PSUM"))
                ps_l = p2.enter_context(
                    tc.tile_pool(name="psl", bufs=2, space="PSUM"))
                ps_p = p2.enter_context(
                    tc.tile_pool(name="psp", bufs=2, space="PSUM"))
                for qc in reversed(range(NQC)):
                    q0 = qc * QC
                    nsb = (qc + 1) * (QC // 128)  # causal: s-blocks <= q-chunk end
                    for h in range(HPC):
                        po = ps_o.tile([128, QC], f32, tag="pso")
                        pl = ps_l.tile([128, QC], f32, tag="psl")
                        for sb in range(nsb):
                            s0 = sb * 128
                            # causal: only q-columns >= s0 are live in this
                            # s-block; skip the dead prefix entirely
                            j0 = max(0, s0 - q0)
                            ps = ps_s.tile([128, QC], f32, tag="pss")
                            nc.tensor.matmul(
                                ps[:, j0:QC],
                                kT[h][:, s0:s0 + 128],
                                qT[h][:, q0 + j0:q0 + QC],
                                start=True, stop=True)
                            et = etp.tile([128, QC], bf16, tag="et")
                            nc.scalar.activation(
                                et[:, j0:QC], ps[:, j0:QC],
                                mybir.ActivationFunctionType.Exp, scale=SCALE)
                            if s0 >= q0:  # intra-block triangle mask
                                nc.vector.tensor_mul(
                                    et[:, j0:j0 + 128], et[:, j0:j0 + 128],
                                    tri_sb)
                            nc.tensor.matmul(
                                po[:, j0:QC], vP[sb][:, h * HD:(h + 1) * HD],
                                et[:, j0:QC],
                                start=(sb == 0), stop=(sb == nsb - 1))
                            nc.tensor.matmul(
                                pl[:, j0:QC], ones_sb, et[:, j0:QC],
                                start=(sb == 0), stop=(sb == nsb - 1))
                        linv = lvp.tile([128, QC], f32, tag="linv")
                        nc.vector.reciprocal_approx_fast(out=linv, in_=pl[:, :])
                        nc.vector.tensor_mul(
                            aT[h][:, q0:q0 + QC], po[:, :], linv)
                    # P3 for this q-chunk's t-range; ec-pairs share one
                    # [128, 1024] staging tile and a single store DMA
                    for tb in range(QC // 128):
                        t0 = q0 + tb * 128
                        for ecp in range(2):
                            ot = op.tile([128, 1024], f32, tag="ot")
                            for eh in range(2):
                                ec = ecp * 2 + eh
                                psp = ps_p.tile([128, 512], f32, tag="psp")
                                for h in range(HPC):
                                    nc.tensor.matmul(
                                        psp[:, :],
                                        aT[h][:, t0:t0 + 128],
                                        wp_sb[:, h * D + ec * 512:
                                              h * D + (ec + 1) * 512],
                                        start=(h == 0), stop=(h == HPC - 1))
                                if eh == 0:
                                    nc.scalar.copy(
                                        ot[:, 0:512], psp[:, :])
                                else:
                                    nc.vector.tensor_copy(
                                        ot[:, 512:1024], psp[:, :])
                            nc.sync.dma_start(
                                out=outp[t0:t0 + 128,
                                         ecp * 1024:(ecp + 1) * 1024],
                                in_=ot)
    nc.finalize()
    return nc


def _prep_in_maps(x, rope, mask, w_attn, w_proj):
    import ml_dtypes
    bf = ml_dtypes.bfloat16
    x = np.asarray(x, dtype=np.float32)
    rope = np.asarray(rope, dtype=np.float32)
    mask = np.asarray(mask)
    w_attn = np.asarray(w_attn, dtype=np.float32)
    w_proj = np.asarray(w_proj, dtype=np.float32)

    xTb = [np.ascontiguousarray(x[b].T).astype(bf) for b in range(B)]
    cosT = np.ascontiguousarray(rope[:, :, 0].T)  # (64, T)
    sinT = np.ascontiguousarray(rope[:, :, 1].T)
    cos2 = np.concatenate([cosT, cosT], axis=0).astype(bf)    # (128, T)
    sinn = np.concatenate([-sinT, sinT], axis=0).astype(bf)   # (128, T)

    # intra-block triangle: tri[i, j] = mask[q=j, s=i] over a 128 window
    m128 = np.asarray(mask[0, 0, :128, :128])
    dm = np.ascontiguousarray(m128.T.astype(np.float32)).astype(bf)

    perm = np.concatenate([np.arange(0, HD, 2), np.arange(1, HD, 2)])
    in_maps = []
    for c in range(8):
        b = c // 4
        heads = [4 * (c % 4) + j for j in range(HPC)]
        qrows = np.concatenate([w_attn[h * HD:(h + 1) * HD][perm] for h in heads])
        krows = np.concatenate(
            [w_attn[D + h * HD:D + (h + 1) * HD][perm] for h in heads])
        vrows = np.concatenate(
            [w_attn[2 * D + h * HD:2 * D + (h + 1) * HD] for h in heads])
        wqkv_m = np.ascontiguousarray(
            np.concatenate([qrows, krows, vrows]).T).astype(bf)  # (D, 1536)
        wp_m = np.ascontiguousarray(np.concatenate(
            [w_proj[:, h * HD:(h + 1) * HD].T for h in heads],
            axis=1)).astype(bf)                                  # (128, 4*D)
        im = {"xT": xTb[b], "wqkv": wqkv_m, "wp": wp_m,
              "cossin": np.concatenate([cos2, sinn], axis=1),
              "trione": np.concatenate(
                  [dm, np.ones((128, 128), np.float32).astype(bf)], axis=1),
              "tick": np.zeros((128, 8), np.float32)}
        in_maps.append(im)
    return in_maps


def kernel(x, rope, mask, w_attn, w_proj):
    if "nc" not in _BUILT:
        _BUILT["nc"] = _build_nc()
    nc = _BUILT["nc"]
    in_maps = _prep_in_maps(x, rope, mask, w_attn, w_proj)
    res = run_bass_kernel_spmd(nc, in_maps, core_ids=list(range(8)))
    out = np.zeros((B, T, D), dtype=np.float64)
    for c in range(8):
        out[c // 4] += res.results[c]["outp"].astype(np.float64)
    return out.astype(np.float32)


# revision 22
# speedup vs baseline: 1.1351x; 1.0401x over previous
"""Causal self-attention (B=2, T=2048, D=2048, H=16) on 8 TRN2 NeuronCores.

Sharding: 2-way batch-parallel x 4-way head-parallel. Core c owns batch
c//4 and heads 4*(c%4)..4*(c%4)+3. Each core computes its heads'
contribution through the output projection for its batch; the host sums
the 4 partials per batch.

Per-core kernel (bf16 compute, f32 PSUM accumulation):
  P1  QKV projection over t-chunks of 512: Q^T/K^T produced in [hd, t]
      layout (rope "split" order: re pairs in partitions 0-63, im in
      64-127) via fused-wqkv slices as stationary; rope applied with 4
      DVE ops reading the PSUM tile directly; V natural [t, hd] per
      head (ACT copy). x loads are chunk-pair batched [128, 1024] and
      ride the scalar-engine HWDGE ring so they issue in parallel with
      the sync-ring weight loads at startup.
  P2  Flash-style causal attention per (q-chunk, head) in S^T layout:
      S^T = K_blk @ Q^T (PSUM), E^T = exp(scale*S^T) via ACT (bf16),
      then two accumulating matmuls per s-block: O^T += V_blk.T @ E^T
      (N=512, output already transposed) and L += ones.T @ E^T
      (all-ones stationary -> every partition row of L is the softmax
      denominator, i.e. a free broadcast). Diagonal s-blocks compute
      only the live q-column suffix and apply a single reusable 128x128
      triangle mask. Normalize: reciprocal_approx_fast(L), then
      O^T * (1/L) -> attnT.
  P3  Output projection interleaved per q-chunk: attnT x wp with 4
      head-blocks as contraction steps; PSUM->SBUF copies split between
      ACT and DVE; stores overlap compute.
"""
import numpy as np

import concourse.bass as bass
from concourse import bacc
import concourse.tile as tile
from concourse import mybir
from concourse.bass_utils import run_bass_kernel_spmd

B, T, D, H = 2, 2048, 2048, 16
HD = D // H            # 128
HPC = 4                # heads per core
FL = HPC * HD          # local features = 512
TC = 512               # t-chunk for QKV phase
NTC = T // TC          # 4
QC = 512               # q-chunk for attention
NQC = T // QC          # 4
SCALE = float(1.0 / np.sqrt(np.float32(HD)))

f32 = mybir.dt.float32
bf16 = mybir.dt.bfloat16

_BUILT = {}


def _build_nc():
    nc = bacc.Bacc()

    xT = nc.dram_tensor("xT", (D, T), bf16, kind="ExternalInput")
    wqkv = nc.dram_tensor("wqkv", (D, 3 * FL), bf16, kind="ExternalInput")
    wp = nc.dram_tensor("wp", (HD, HPC * D), bf16, kind="ExternalInput")
    cossin = nc.dram_tensor("cossin", (HD, 2 * T), bf16, kind="ExternalInput")
    trione = nc.dram_tensor("trione", (128, 256), bf16, kind="ExternalInput")
    outp = nc.dram_tensor("outp", (T, D), f32, kind="ExternalOutput")
    tick = nc.dram_tensor("tick", (128, 8), f32, kind="ExternalInput")
    tock = nc.dram_tensor("tock", (128, 8), f32, kind="ExternalOutput")

    with tile.TileContext(nc) as tc:
        from contextlib import ExitStack
        with ExitStack() as top:
            pers = top.enter_context(tc.tile_pool(name="pers", bufs=1))
            # persistent: weights, rope multipliers, masks, ones
            wqkv_sb = [pers.tile([128, 3 * FL], bf16, tag=f"wqkv{k}",
                                 name=f"wqkv{k}") for k in range(16)]
            cs_sb = pers.tile([HD, 2 * T], bf16)
            wp_sb = pers.tile([128, HPC * D], bf16)
            to_sb = pers.tile([128, 256], bf16)
            # per-core persistent activations
            qT = [pers.tile([HD, T], bf16, tag=f"qT{h}", name=f"qT{h}")
                  for h in range(HPC)]
            kT = [pers.tile([HD, T], bf16, tag=f"kT{h}", name=f"kT{h}")
                  for h in range(HPC)]
            vP = [pers.tile([128, FL], bf16, tag=f"vP{i}", name=f"vP{i}")
                  for i in range(T // 128)]
            aT = [pers.tile([128, T], bf16, tag=f"aT{h}", name=f"aT{h}")
                  for h in range(HPC)]

            # DMA order matters for startup: fused wqkv + cos/sin first
            # (P1 chunk 0 needs them); wp/masks last (P2/P3 only).
            for k in range(16):
                nc.sync.dma_start(out=wqkv_sb[k],
                                  in_=wqkv[k * 128:(k + 1) * 128, :])
            nc.sync.dma_start(out=cs_sb, in_=cossin[:, :])
            tick_sb = pers.tile([128, 8], f32)
            nc.sync.dma_start(out=tick_sb, in_=tick[:, :])
            nc.sync.dma_start(out=tock[:, :], in_=tick_sb)
            nc.sync.dma_start(out=wp_sb, in_=wp[:, :])
            nc.sync.dma_start(out=to_sb, in_=trione[:, :])
            tri_sb = to_sb[:, 0:128]
            ones_sb = to_sb[:, 128:256]

            # ---- cascaded pipeline: per chunk c, QKV(c) then
            # attention+projection for q-chunk c (needs only chunks <= c).
            # Single scope so the scheduler can interleave phases: exp/DVE
            # work spreads into the matmul-dense QKV stretches.
            with ExitStack() as p1:
                xp = p1.enter_context(tc.tile_pool(name="xp", bufs=20))
                rp = p1.enter_context(tc.tile_pool(name="rp", bufs=4))
                etp = p1.enter_context(tc.tile_pool(name="etp", bufs=6))
                lvp = p1.enter_context(tc.tile_pool(name="lvp", bufs=3))
                op = p1.enter_context(tc.tile_pool(name="op", bufs=3))
                ps_qk = p1.enter_context(
                    tc.tile_pool(name="psqk", bufs=2, space="PSUM"))
                ps_v = p1.enter_context(
                    tc.tile_pool(name="psv", bufs=1, space="PSUM"))
                ps_s = p1.enter_context(
                    tc.tile_pool(name="pss", bufs=2, space="PSUM"))
                ps_o = p1.enter_context(
                    tc.tile_pool(name="pso", bufs=2, space="PSUM"))
                ps_l = p1.enter_context(
                    tc.tile_pool(name="psl", bufs=1, space="PSUM"))
                for t in range(NTC):
                    t0 = t * TC
                    xt = []
                    for k in range(16):
                        xk = xp.tile([128, TC], bf16, tag="xt", name="xt")
                        # scalar-ring HWDGE: parallel issue with the
                        # sync-ring weight loads at startup
                        nc.scalar.dma_start(
                            out=xk, in_=xT[k * 128:(k + 1) * 128, t0:t0 + TC])
                        xt.append(xk)
                    # Q^T/K^T accumulate: 8 f-blocks (q_h0..q_h3, k_h0..k_h3)
                    for fb in range(8):
                        ps = ps_qk.tile([128, TC], f32, tag="psqk")
                        for k in range(16):
                            nc.tensor.matmul(
                                ps[:, :],
                                wqkv_sb[k][:, fb * 128:(fb + 1) * 128],
                                xt[k][:, :],
                                start=(k == 0), stop=(k == 15))
                        # rope from psum -> SBUF qT/kT slice
                        dst = (qT[fb] if fb < HPC else kT[fb - HPC])
                        dsl = dst[:, t0:t0 + TC]
                        ca = cs_sb[:, t0:t0 + TC]
                        sa = cs_sb[:, T + t0:T + t0 + TC]
                        m1 = rp.tile([128, TC], bf16, tag="m1")
                        m2 = rp.tile([128, TC], bf16, tag="m2")
                        nc.vector.tensor_mul(m1, ps[:, :], ca)
                        nc.vector.tensor_mul(
                            m2[0:64, :], ps[64:128, :], sa[0:64, :])
                        nc.vector.tensor_mul(
                            m2[64:128, :], ps[0:64, :], sa[64:128, :])
                        nc.vector.tensor_add(dsl, m1, m2)
                    # V natural accumulate: TC/128 t-blocks
                    for tb in range(TC // 128):
                        ps = ps_v.tile([128, FL], f32, tag="psv")
                        for k in range(16):
                            nc.tensor.matmul(
                                ps[:, :],
                                xt[k][:, tb * 128:(tb + 1) * 128],
                                wqkv_sb[k][:, 2 * FL:3 * FL],
                                start=(k == 0), stop=(k == 15))
                        nc.scalar.copy(vP[(t0 + tb * 128) // 128], ps[:, :])

                    # attention + projection for q-chunk t
                    qc = t
                    q0 = qc * QC
                    nsb = (qc + 1) * (QC // 128)  # causal: s-blocks <= chunk end
                    for h in range(HPC):
                        po = ps_o.tile([128, QC], f32, tag="pso")
                        pl = ps_l.tile([128, QC], f32, tag="psl")
                        for sb in range(nsb):
                            s0 = sb * 128
                            # causal: only q-columns >= s0 are live in this
                            # s-block; skip the dead prefix entirely
                            j0 = max(0, s0 - q0)
                            ps = ps_s.tile([128, QC], f32, tag="pss")
                            nc.tensor.matmul(
                                ps[:, j0:QC],
                                kT[h][:, s0:s0 + 128],
                                qT[h][:, q0 + j0:q0 + QC],
                                start=True, stop=True)
                            et = etp.tile([128, QC], bf16, tag="et")
                            nc.scalar.activation(
                                et[:, j0:QC], ps[:, j0:QC],
                                mybir.ActivationFunctionType.Exp, scale=SCALE)
                            if s0 >= q0:  # intra-block triangle mask
                                nc.vector.tensor_mul(
                                    et[:, j0:j0 + 128], et[:, j0:j0 + 128],
                                    tri_sb)
                            nc.tensor.matmul(
                                po[:, j0:QC], vP[sb][:, h * HD:(h + 1) * HD],
                                et[:, j0:QC],
                                start=(sb == 0), stop=(sb == nsb - 1))
                            nc.tensor.matmul(
                                pl[:, j0:QC], ones_sb, et[:, j0:QC],
                                start=(sb == 0), stop=(sb == nsb - 1))
                        linv = lvp.tile([128, QC], f32, tag="linv")
                        nc.vector.reciprocal_approx_fast(out=linv, in_=pl[:, :])
                        nc.vector.tensor_mul(
                            aT[h][:, q0:q0 + QC], po[:, :], linv)
                    # projection for this q-chunk's t-range; ec-pairs share
                    # one [128, 1024] staging tile and a single store DMA
                    for tb in range(QC // 128):
                        t0p = q0 + tb * 128
                        for ecp in range(2):
                            ot = op.tile([128, 1024], f32, tag="ot")
                            for eh in range(2):
                                ec = ecp * 2 + eh
                                psp = ps_o.tile([128, 512], f32, tag="pso")
                                for h in range(HPC):
                                    nc.tensor.matmul(
                                        psp[:, :],
                                        aT[h][:, t0p:t0p + 128],
                                        wp_sb[:, h * D + ec * 512:
                                              h * D + (ec + 1) * 512],
                                        start=(h == 0), stop=(h == HPC - 1))
                                if eh == 0:
                                    nc.scalar.copy(
                                        ot[:, 0:512], psp[:, :])
                                else:
                                    nc.vector.tensor_copy(
                                        ot[:, 512:1024], psp[:, :])
                            nc.sync.dma_start(
                                out=outp[t0p:t0p + 128,
                                         ecp * 1024:(ecp + 1) * 1024],
                                in_=ot)
    nc.finalize()
    return nc


def _prep_in_maps(x, rope, mask, w_attn, w_proj):
    import ml_dtypes
    bf = ml_dtypes.bfloat16
    x = np.asarray(x, dtype=np.float32)
    rope = np.asarray(rope, dtype=np.float32)
    mask = np.asarray(mask)
    w_attn = np.asarray(w_attn, dtype=np.float32)
    w_proj = np.asarray(w_proj, dtype=np.float32)

    xTb = [np.ascontiguousarray(x[b].T).astype(bf) for b in range(B)]
    cosT = np.ascontiguousarray(rope[:, :, 0].T)  # (64, T)
    sinT = np.ascontiguousarray(rope[:, :, 1].T)
    cos2 = np.concatenate([cosT, cosT], axis=0).astype(bf)    # (128, T)
    sinn = np.concatenate([-sinT, sinT], axis=0).astype(bf)   # (128, T)

    # intra-block triangle: tri[i, j] = mask[q=j, s=i] over a 128 window
    m128 = np.asarray(mask[0, 0, :128, :128])
    dm = np.ascontiguousarray(m128.T.astype(np.float32)).astype(bf)

    perm = np.concatenate([np.arange(0, HD, 2), np.arange(1, HD, 2)])
    in_maps = []
    for c in range(8):
        b = c // 4
        heads = [4 * (c % 4) + j for j in range(HPC)]
        qrows = np.concatenate([w_attn[h * HD:(h + 1) * HD][perm] for h in heads])
        krows = np.concatenate(
            [w_attn[D + h * HD:D + (h + 1) * HD][perm] for h in heads])
        vrows = np.concatenate(
            [w_attn[2 * D + h * HD:2 * D + (h + 1) * HD] for h in heads])
        wqkv_m = np.ascontiguousarray(
            np.concatenate([qrows, krows, vrows]).T).astype(bf)  # (D, 1536)
        wp_m = np.ascontiguousarray(np.concatenate(
            [w_proj[:, h * HD:(h + 1) * HD].T for h in heads],
            axis=1)).astype(bf)                                  # (128, 4*D)
        im = {"xT": xTb[b], "wqkv": wqkv_m, "wp": wp_m,
              "cossin": np.concatenate([cos2, sinn], axis=1),
              "trione": np.concatenate(
                  [dm, np.ones((128, 128), np.float32).astype(bf)], axis=1),
              "tick": np.zeros((128, 8), np.float32)}
        in_maps.append(im)
    return in_maps


def kernel(x, rope, mask, w_attn, w_proj):
    if "nc" not in _BUILT:
        _BUILT["nc"] = _build_nc()
    nc = _BUILT["nc"]
    in_maps = _prep_in_maps(x, rope, mask, w_attn, w_proj)
    res = run_bass_kernel_spmd(nc, in_maps, core_ids=list(range(8)))
    out = np.zeros((B, T, D), dtype=np.float64)
    for c in range(8):
        out[c // 4] += res.results[c]["outp"].astype(np.float64)
    return out.astype(np.float32)
